# revision 20
# baseline (speedup 1.0000x reference)
"""Trainium2 Bass kernel for MultiLinearAttention (causal linear attention).

Reference computation (per head h, feature map phi(u) = elu(u)+1):
    q = phi(x_h @ Wq_h), k = phi(x_h @ Wk_h), v = x_h @ Wv_h
    y_t = (q_t . sum_{s<=t} k_s v_s^T) / (q_t . sum_{s<=t} k_s + eps)
    out = concat_h(y_h) @ Wp

Sharding: 16 heads / 8 cores = 2 heads per core, all 4 batches per core.
Wp is folded per-head into the v projection (W'_h = Wv_h @ Wp_h); each core
ships per-head numerators [B, S, 2, 64] and denominators [B, S, 2]; the
host unshard computes y = sum_cores sum_h num_h / (den_h + eps).

Device algorithm: chunked causal linear attention, chunk C=128, all 4
batches fused per chunk into wide ops:
    u = 1 + [q|k] projections (PSUM preset via K=1 ones matmul)
    phi = max(u, min(exp(u-1), 1)) == elu(.)+1
    A^T = K Q^T per (b,h) (8 blocks, h-major: [4xh0 | 4xh1])
    am = A ⊙ causal-mask (one DVE op over all 8 blocks)
    num = am^T V + Q S_prev; den = am^T 1 + Q z_prev
    S += Kt^T V (PSUM-persistent); z via PSUM chunk-sums + SBUF f32 acc.

PSUM banks (8): state[512]f32 | u[1024]f32 x2 | A[1024]f32 x2 |
num[512]f32 | vk[512]f32 | {knp[512]bf16 + den/z [12]f32} shared.
"""

import os
import sys

import numpy as np

for _p in ("/root/.axon_site/_ro/trn_rl_repo", "/opt/trn_rl_repo", "/opt/pypackages"):
    if os.path.isdir(_p) and _p not in sys.path:
        sys.path.append(_p)

import ml_dtypes

B, S, D = 4, 4096, 1024
H, HD, O = 16, 64, 64
C = 128                  # chunk length
NCORE = 8
HPC = H // NCORE         # heads per core
NCHUNK = S // C
EPS = 1e-6

_CACHE = {}


def _build_program(nchunk=NCHUNK):
    import concourse.mybir as mybir
    from concourse import bacc
    from concourse.tile import TileContext

    fp32 = mybir.dt.float32
    cdt = mybir.dt.bfloat16
    Alu = mybir.AluOpType
    Act = mybir.ActivationFunctionType

    nc = bacc.Bacc()
    # x staged feature-major, chunk-interleaved: col = 512*chunk + 128*b + s
    xq_h = nc.declare_dram_parameter("xq", [128, 4 * S], cdt, isOutput=False)
    wq_h = nc.declare_dram_parameter("wq", [128, 128], cdt, isOutput=False)
    wk_h = nc.declare_dram_parameter("wk", [128, 128], cdt, isOutput=False)
    wv_h = nc.declare_dram_parameter("wv", [128, 128], cdt, isOutput=False)
    mask_h = nc.declare_dram_parameter("mask8", [128, 1024], cdt, isOutput=False)
    ident_h = nc.declare_dram_parameter("ident", [128, 128], cdt, isOutput=False)
    ones_h = nc.declare_dram_parameter("ones", [1, 512], cdt, isOutput=False)
    zer_h = nc.declare_dram_parameter("zer", [1, 512], cdt, isOutput=False)
    out_h = nc.declare_dram_parameter("out", [B, S, 128], fp32, isOutput=True)
    den_h = nc.declare_dram_parameter("den", [B, S, 2], fp32, isOutput=True)

    NXT = 8               # number of x SBUF tiles
    XCOLS = 4 * S // NXT  # 2048 cols per tile = 4 chunks

    with TileContext(nc) as tc:
        with (
            tc.tile_pool(name="consts", bufs=1) as consts,
            tc.tile_pool(name="work", bufs=2) as work,
            tc.tile_pool(name="stage", bufs=2) as stage,
            # PSUM pools; creation order fixes bank layout (8 banks total)
            tc.tile_pool(name="pu", bufs=3, space="PSUM") as pu,
            tc.tile_pool(name="pa", bufs=1, space="PSUM") as pa,
            tc.tile_pool(name="pn", bufs=1, space="PSUM") as pn,
            tc.tile_pool(name="psv", bufs=1, space="PSUM") as psv,
            tc.tile_pool(name="pkz", bufs=1, space="PSUM") as pkz,
        ):
            # ---- constants into SBUF ----
            wq = consts.tile([128, 128], cdt)
            wk = consts.tile([128, 128], cdt)
            wv = consts.tile([128, 128], cdt)
            mask8 = consts.tile([128, 1024], cdt)
            ident = consts.tile([128, 128], cdt)
            ones = consts.tile([1, 512], cdt)
            zer = consts.tile([1, 512], cdt)
            # SWDGE (gpsimd) for input loads keeps the SP sequencer free for
            # the batched output DMAs.
            nc.gpsimd.dma_start(wq, wq_h[:, :])
            nc.gpsimd.dma_start(wk, wk_h[:, :])
            nc.gpsimd.dma_start(wv, wv_h[:, :])
            nc.gpsimd.dma_start(mask8, mask_h[:, :])
            nc.gpsimd.dma_start(ident, ident_h[:, :])
            nc.gpsimd.dma_start(ones, ones_h[:, :])
            nc.gpsimd.dma_start(zer, zer_h[:, :])

            ones_col = consts.tile([128, 1], cdt)
            nc.gpsimd.memset(ones_col, 1.0)
            neg1 = consts.tile([128, 1], fp32)
            nc.gpsimd.memset(neg1, -1.0)

            xsb = []
            for t in range(NXT):
                xt = consts.tile([128, XCOLS], cdt, name=f"xsb{t}")
                nc.gpsimd.dma_start(xt, xq_h[:, t * XCOLS:(t + 1) * XCOLS])
                xsb.append(xt)

            def xchunk(i):
                """[128, 512] x columns of chunk i (4 batches)."""
                t, r = divmod(i * 512, XCOLS)
                return xsb[t][:, r:r + 512]

            # ping-pong SBUF state copies for den path
            s01z = [consts.tile([128, 8], cdt, name=f"s01z{j}") for j in range(2)]
            for t in s01z:
                nc.gpsimd.memset(t, 0.0)
            zsum = [consts.tile([128, 4], fp32, name=f"zsum{j}") for j in range(2)]

            # ---- persistent [state | vk] PSUM tile (2 banks) ----
            # state in bank A (accumulates forever, zeroed once); vk in bank
            # B (rewritten per chunk, start=True clears only its own bank).
            # One wide Act copy evacuates both as [s01v | vsb] bf16.
            sv = psv.tile([128, 1024], fp32, name="sv")
            state = sv[:, 0:512]
            vkreg = sv[:, 512:1024]
            nc.tensor.matmul(state, ones[:, 0:128], zer[:, 0:512],
                             start=True, stop=False, skip_group_check=True)

            def emit_uhalf(i, w):
                """preset + one projection (q or k) for chunk i -> [128,512].
                Separate q/k tiles double-buffer the u banks, breaking the
                phi(i) -> proj(i+1) -> exp(i+1) -> phi(i+1) serial ring."""
                xc = xchunk(i)
                u = pu.tile([128, 512], fp32, name="u", tag="u")
                nc.tensor.matmul(u, ones[:, 0:128], ones[:, 0:512],
                                 start=True, stop=False, skip_group_check=True)
                nc.tensor.matmul(u, w, xc, start=False, stop=True,
                                 skip_group_check=True)
                return u

            def emit_v(i):
                xc = xchunk(i)
                for b in range(4):
                    nc.tensor.matmul(vkreg[:, 128 * b:128 * (b + 1)],
                                     xc[:, 128 * b:128 * (b + 1)], wv,
                                     start=(b == 0), stop=(b == 3),
                                     skip_group_check=True)

            def emit_phi_half(u, nm):
                """phi(u) = max(u, min(exp(u-1), 1)); u holds proj+1."""
                e2 = work.tile([128, 512], cdt, name=f"e2{nm}", tag=f"e2{nm}")
                nc.scalar.activation(e2, u, Act.Exp, bias=neg1)
                ph = work.tile([128, 512], cdt, name=f"ph{nm}", tag=f"ph{nm}")
                nc.vector.scalar_tensor_tensor(ph, e2, 1.0, u, Alu.min, Alu.max)
                return ph

            # ---- prologue: chunk 0 front ----
            uk = emit_uhalf(0, wk)
            uq = emit_uhalf(0, wq)
            emit_v(0)
            phik = emit_phi_half(uk, "k")
            phiq = emit_phi_half(uq, "q")
            comb = work.tile([128, 1024], cdt, name="comb")
            nc.scalar.copy(comb[:, 512:1024], vkreg)
            s01v_prev = None
            vsb = comb[:, 512:1024]
            numwide = denwide = None
            BCH = 8               # chunks per output-DMA batch

            for i in range(nchunk):
                if i % BCH == 0:
                    numwide = stage.tile([128, 512 * BCH], fp32, name="numwide")
                    denwide = stage.tile([128, 8 * BCH], fp32, name="denwide")

                # ---- next-chunk projections first: feeds exp->phi chain ----
                if i + 1 < nchunk:
                    uk = emit_uhalf(i + 1, wk)
                    uq = emit_uhalf(i + 1, wq)

                # ---- A^T = K Q^T per (b,h); one bank, per-head halves ----
                am_h = []
                for h in range(2):
                    es = slice(64 * h, 64 * (h + 1))
                    a_ps = pa.tile([128, 512], fp32, name="a_ps", tag="a")
                    for b in range(4):
                        nc.tensor.matmul(
                            a_ps[:, 128 * b:128 * (b + 1)],
                            phik[es, 128 * b:128 * (b + 1)],
                            phiq[es, 128 * b:128 * (b + 1)],
                            start=True, stop=True)
                    amh = work.tile([128, 512], cdt, name=f"am{h}",
                                    tag=f"am{h}")
                    nc.vector.tensor_tensor(amh, a_ps, mask8[:, 0:512],
                                            Alu.mult)
                    am_h.append(amh)

                if i + 1 < nchunk:
                    emit_v(i + 1)

                # ---- transpose phi(k) per batch -> token-major (bf16 PSUM) --
                # knp shares its bank with den/z; transposes must precede the
                # den writers of this chunk (PE order does that).
                kdz = pkz.tile([128, 544], cdt, name="kdz")
                knp = kdz[:, 0:512]
                denz = kdz[:, 512:536].bitcast(fp32)   # [128, 12] f32
                for b in range(4):
                    nc.tensor.transpose(
                        knp[:, 128 * b:128 * (b + 1)],
                        phik[:, 128 * b:128 * (b + 1)], ident)
                knat = work.tile([128, 512], cdt, name="knat")
                nc.vector.tensor_copy(knat.bitcast(fp32), knp.bitcast(fp32))

                num = pn.tile([128, 512], fp32, name="num")

                # ---- cross-chunk terms: Q S_prev, Q z_prev ----
                if i > 0:
                    for b in range(4):
                        nc.tensor.matmul(
                            num[:, 128 * b:128 * (b + 1)],
                            phiq[:, 128 * b:128 * (b + 1)],
                            s01v_prev[:, 128 * b:128 * (b + 1)],
                            start=(b == 0), stop=False, skip_group_check=True)
                    for b in range(4):
                        nc.tensor.matmul(
                            denz[:, 2 * b:2 * b + 2],
                            phiq[:, 128 * b:128 * (b + 1)],
                            s01z[(i - 1) % 2][:, 2 * b:2 * b + 2],
                            start=(b == 0), stop=False, skip_group_check=True)

                # exp/phi for next chunk (Act+DVE overlap with PE below)
                if i + 1 < nchunk:
                    phik_n = emit_phi_half(uk, "k")
                    phiq_n = emit_phi_half(uq, "q")
                else:
                    phiq_n = phik_n = None

                # ---- intra-chunk: num += am^T V, den += am^T 1 ----
                for h in range(2):
                    for b in range(4):
                        amb = am_h[h][:, 128 * b:128 * (b + 1)]
                        nc.tensor.matmul(
                            num[:, 128 * b + 64 * h:128 * b + 64 * (h + 1)],
                            amb, vsb[:, 128 * b + 64 * h:128 * b + 64 * (h + 1)],
                            start=(i == 0 and h == 0 and b == 0), stop=True,
                            skip_group_check=True)
                        nc.tensor.matmul(
                            denz[:, 2 * b + h:2 * b + h + 1],
                            amb, ones_col,
                            start=(i == 0 and h == 0 and b == 0), stop=False,
                            skip_group_check=True)

                # ---- state update: S += Kt^T V; z_chunk = Kt^T 1 ----
                for h in range(2):
                    for b in range(4):
                        kt = knat[:, 128 * b + 64 * h:128 * b + 64 * (h + 1)]
                        nc.tensor.matmul(
                            state[64 * h:64 * (h + 1),
                                  128 * b + 64 * h:128 * b + 64 * (h + 1)],
                            kt, vsb[:, 128 * b + 64 * h:128 * b + 64 * (h + 1)],
                            start=False, stop=False, skip_group_check=True)
                for b in range(4):
                    nc.tensor.matmul(
                        denz[:, 8 + b:9 + b],
                        knat[:, 128 * b:128 * (b + 1)], ones_col,
                        start=False, stop=(b == 3), skip_group_check=True)

                # ---- [state | vk] -> SBUF for next chunk's cross terms ----
                if i + 1 < nchunk:
                    comb_n = work.tile([128, 1024], cdt, name="comb")
                    nc.scalar.copy(comb_n, sv)
                    s01v_prev = comb_n[:, 0:512]
                    vsb_n = comb_n[:, 512:1024]
                    zc = denz[:, 8:12]
                    if i == 0:
                        nc.vector.tensor_copy(zsum[0], zc)
                    else:
                        nc.vector.tensor_tensor(zsum[i % 2], zc,
                                                zsum[(i - 1) % 2], Alu.add)
                    szt = s01z[i % 2]
                    szv = szt.rearrange("p (g c) -> p g c", c=2)
                    zs = zsum[i % 2]
                    nc.gpsimd.tensor_copy(szv[0:64, :, 0:1], zs[0:64, :])
                    nc.gpsimd.tensor_copy(szv[64:128, :, 1:2], zs[64:128, :])
                else:
                    vsb_n = None

                # ---- evacuate num/den into wide staging; DMA every BCH ----
                ci = i % BCH
                nc.scalar.copy(numwide[:, 512 * ci:512 * (ci + 1)], num)
                nc.vector.tensor_copy(denwide[:, 8 * ci:8 * (ci + 1)],
                                      denz[:, 0:8])
                if ci == BCH - 1:
                    blk = slice((i - ci) * C, (i + 1) * C)
                    nwv = numwide.rearrange("p (c bo) -> p c bo", bo=512)
                    dwv = denwide.rearrange("p (c d) -> p c d", d=8)
                    for b in range(4):
                        nc.sync.dma_start(
                            out_h[b, blk, :].rearrange("(c s) o -> s c o",
                                                       s=128),
                            nwv[:, :, 128 * b:128 * (b + 1)])
                        nc.sync.dma_start(
                            den_h[b, blk, :].rearrange("(c s) d -> s c d",
                                                       s=128),
                            dwv[:, :, 2 * b:2 * b + 2])

                phiq = phiq_n
                phik = phik_n
                vsb = vsb_n

    nc.finalize()
    return nc


def _host_prep(x, Wq, Wk, Wv, Wp):
    """Shard inputs per core; returns in_maps list."""
    x = np.asarray(x, dtype=np.float32)
    Wq = np.asarray(Wq, dtype=np.float32)
    Wk = np.asarray(Wk, dtype=np.float32)
    Wv = np.asarray(Wv, dtype=np.float32)
    Wp = np.asarray(Wp, dtype=np.float32)
    ndt = ml_dtypes.bfloat16

    mask = np.triu(np.ones((C, C), np.float32))
    mask8 = np.tile(mask, (1, 8)).astype(ndt)          # [128, 1024]
    ident = np.eye(128, dtype=np.float32).astype(ndt)
    ones = np.ones((1, 512), np.float32).astype(ndt)
    zer = np.zeros((1, 512), np.float32).astype(ndt)

    in_maps = []
    for c in range(NCORE):
        h0 = HPC * c
        xs = x[:, :, 64 * h0:64 * (h0 + HPC)]          # [B, S, 128]
        xT = xs.transpose(2, 0, 1)                     # [128, B, S]
        # col = 512*chunk + 128*b + s_in_chunk
        xq = np.ascontiguousarray(
            xT.reshape(128, B, NCHUNK, C).transpose(0, 2, 1, 3)
        ).reshape(128, B * S).astype(ndt)
        wq_bd = np.zeros((128, 128), np.float32)
        wk_bd = np.zeros((128, 128), np.float32)
        wv_bd = np.zeros((128, 128), np.float32)
        for j in range(HPC):
            h = h0 + j
            sl = slice(64 * j, 64 * (j + 1))
            wq_bd[sl, sl] = Wq[h]
            wk_bd[sl, sl] = Wk[h]
            wv_bd[sl, sl] = Wv[h] @ Wp[64 * h:64 * (h + 1), :]
        in_maps.append({
            "xq": xq,
            "wq": wq_bd.astype(ndt),
            "wk": wk_bd.astype(ndt),
            "wv": wv_bd.astype(ndt),
            "mask8": mask8,
            "ident": ident,
            "ones": ones,
            "zer": zer,
        })
    return in_maps


def get_program():
    if "nc" not in _CACHE:
        _CACHE["nc"] = _build_program()
    return _CACHE["nc"]


def run_spmd(in_maps, **kwargs):
    from concourse.bass_utils import run_bass_kernel_spmd
    nc = get_program()
    return run_bass_kernel_spmd(nc, in_maps, list(range(NCORE)), **kwargs)


def kernel(x, Wq, Wk, Wv, Wp):
    in_maps = _host_prep(x, Wq, Wk, Wv, Wp)
    res = run_spmd(in_maps)
    out = np.zeros((B, S, O), np.float32)
    for c in range(NCORE):
        num = res.results[c]["out"]                    # [B, S, 128]
        den = res.results[c]["den"]                    # [B, S, 2]
        out += num[:, :, 0:64] / (den[:, :, 0:1] + EPS)
        out += num[:, :, 64:128] / (den[:, :, 1:2] + EPS)
    return out
